# revision 20
# baseline (speedup 1.0000x reference)
"""AdjStackAttentionWeights kernel for 8 Trainium2 NeuronCores.

Computation: masked BatchNorm (training-mode stats over masked rows of the
whole tensor), normalize, 2-layer MLP (32 -> 64 relu -> 16), mask the output.

Strategy:
  - Shard batch dim b across the 8 cores (data parallel).
  - Host premultiplies x by the mask and lays the result out in the exact
    [128, 2048] SBUF tile layout the device consumes (partition p = q*32+s
    holds feature s of row-quarter q), so every device DMA is a fully
    contiguous 1 MiB read / 512 KiB write.
  - pass 1: bn_stats over [128,512] tile slices -> masked sum / sumsq per
            feature -> AllReduce of [32,2] across cores. The first KEEP
            megatiles stay resident in SBUF and are reused by pass 2.
  - fold:   BN scale folded into W1 (W1' = diag(s)@W1); shift becomes a
            per-partition bias b1' = (beta - mean*s)@W1 + b1 that rides the
            PSUM->SBUF relu copy.
  - pass 2: hT = relu(W1'.T @ xmT + b1'); outT = W2.T @ hT + b2, streamed
            through the PE with 4-way tile_position packing.
  - Matmul operands use float32r (fp32 rounded to 12-bit-less mantissa),
    which streams 4x faster through the PE than plain fp32; the host
    pre-rounds the uploads.  Set USE_F32R = False to fall back to full fp32.
  - Rows with m=0 produce garbage (+b2); the host zeroes them (the reference
    multiplies by the mask anyway).
"""

import numpy as np

B, NN, S, H, HEADS = 8, 512, 32, 64, 16
R_FULL = NN * NN  # 262144 rows per core
FD = 512          # free-dim elements per quarter tile
QS = 4            # quarters stacked on the partition axis
ST = QS * FD      # 2048 rows per supertile
MG = 4            # supertiles per megatile (1 MiB DMA granularity)
NCORES = 8
BN_EPS = 1e-5
USE_F32R = True
KEEP = 12         # megatiles kept resident in SBUF between the passes

_NC_CACHE = {}


def build_nc(ncores=NCORES, rows=R_FULL, keep=KEEP):
    """Build (and bacc-compile) the SPMD bass program for one core."""
    import concourse.bass as bass
    import concourse.tile as tile
    from concourse import bacc, mybir

    f32 = mybir.dt.float32
    fmm = mybir.dt.float32r if USE_F32R else f32
    T = rows // ST          # supertiles
    TG = T // MG            # megatiles
    keep = min(keep, TG)
    assert TG * MG == T and T * ST == rows

    nc = bacc.Bacc("TRN2", target_bir_lowering=False, debug=False,
                   num_devices=ncores)

    xmt = nc.dram_tensor("xmt", [TG, 128, MG * FD], fmm, kind="ExternalInput")
    # w1f: two stacked copies of blockdiag(W1, W1) [64, 128]
    w1f = nc.dram_tensor("w1f", [128, 2 * H], fmm, kind="ExternalInput")
    w2t = nc.dram_tensor("w2t", [128, 2 * HEADS], f32, kind="ExternalInput")
    w1r = nc.dram_tensor("w1r", [S, H], f32, kind="ExternalInput")  # raw W1
    svec = nc.dram_tensor("svec", [S, 4], f32, kind="ExternalInput")
    b1c = nc.dram_tensor("b1c", [H, 1], f32, kind="ExternalInput")
    b2t = nc.dram_tensor("b2t", [128, 1], f32, kind="ExternalInput")
    # padded store: full 128 partitions (pads included) so the store DMA
    # engages all ports with one contiguous 1 MiB write per megatile
    out = nc.dram_tensor("out", [TG, 128, MG * FD], f32,
                         kind="ExternalOutput")

    xview = xmt.ap()
    oview = out.ap()

    with tile.TileContext(nc) as tc:
        with (
            tc.tile_pool(name="wpool", bufs=1) as wpool,
            tc.tile_pool(name="glue", bufs=1) as glue,
            tc.tile_pool(name="bn", bufs=1) as bnpool,
            tc.tile_pool(name="res", bufs=1) as respool,
            tc.tile_pool(name="stream", bufs=5) as stpool,
            tc.tile_pool(name="h", bufs=4) as hpool,
            tc.tile_pool(name="o", bufs=2) as opool,
            tc.tile_pool(name="psum", bufs=2, space="PSUM") as pspool,
            tc.tile_pool(name="psum1", bufs=1, space="PSUM") as pspool1,
            tc.tile_pool(name="dram", bufs=1, space="DRAM") as dpool,
        ):
            # ---- resident weights/constants -------------------------------
            w1sb = wpool.tile([128, 2 * H], fmm)      # 2x blockdiag(W1, W1)
            nc.sync.dma_start(w1sb[:], w1f[:])
            w2sb = wpool.tile([128, 2 * HEADS], f32)  # 2 stacked [64,32] pads
            nc.sync.dma_start(w2sb[:], w2t[:])
            b2sb = wpool.tile([128, 1], f32)
            nc.sync.dma_start(b2sb[:], b2t[:])
            w1rsb = glue.tile([S, H], f32)
            nc.sync.dma_start(w1rsb[:], w1r[:])
            svsb = glue.tile([S, 4], f32)
            nc.sync.dma_start(svsb[:], svec[:])
            b1sb = glue.tile([H, 1], f32)
            nc.sync.dma_start(b1sb[:], b1c[:])

            # ---- pass 1: bn_stats over all tiles --------------------------
            bnbuf = bnpool.tile([128, 6 * T], f32)
            xtiles = {}
            for g in range(TG):
                if g < keep:
                    st_tile = respool.tile([128, MG * FD], fmm, tag=f"res{g}")
                    xtiles[g] = st_tile
                else:
                    st_tile = stpool.tile([128, MG * FD], fmm, tag="stream")
                nc.sync.dma_start(st_tile[:], xview[g])
                for u in range(MG):
                    t = g * MG + u
                    nc.vector.bn_stats(bnbuf[:, 6 * t:6 * t + 6],
                                       st_tile[:, FD * u:FD * u + FD].bitcast(f32))

            # convert (count, mean, count*var) x {even, odd} into sums
            # bnbuf view [128, T, 6]; means at cols 1,4; cvars at cols 2,5
            bnv = bnbuf[:].rearrange("p (t k) -> p t k", k=6)
            means = bnv[:, :, 1:5:3]   # [128, T, 2] (cols 1 and 4)
            cvars = bnv[:, :, 2:6:3]   # [128, T, 2] (cols 2 and 5)
            half = float(FD // 2)

            msq = glue.tile([128, 2 * T], f32)
            nc.vector.tensor_mul(msq[:], means, means)
            sum_means = glue.tile([128, 1], f32)
            nc.vector.tensor_reduce(sum_means[:], means,
                                    axis=mybir.AxisListType.XY,
                                    op=mybir.AluOpType.add)
            sum_msq = glue.tile([128, 1], f32)
            nc.vector.tensor_reduce(sum_msq[:], msq[:],
                                    axis=mybir.AxisListType.X,
                                    op=mybir.AluOpType.add)
            sum_cv = glue.tile([128, 1], f32)
            nc.vector.tensor_reduce(sum_cv[:], cvars,
                                    axis=mybir.AxisListType.XY,
                                    op=mybir.AluOpType.add)
            partials = glue.tile([128, 2], f32)
            # sum(x) = half * sum(means)
            nc.vector.tensor_scalar_mul(partials[:, 0:1], sum_means[:], half)
            # sum(x^2) = half * sum(means^2) + sum(count*var)
            nc.vector.tensor_scalar(partials[:, 1:2], sum_msq[:], half,
                                    sum_cv[:], op0=mybir.AluOpType.mult,
                                    op1=mybir.AluOpType.add)

            # fold the 4 partition quarters: stage [32, 2, 4], reduce over q
            stage = glue.tile([32, 8], f32)
            stv = stage[:].rearrange("s (k q) -> s k q", q=QS)
            for q in range(QS):
                nc.sync.dma_start(stv[:, :, q:q + 1],
                                  partials[32 * q:32 * q + 32, :])
            local = glue.tile([32, 2], f32)
            nc.vector.tensor_reduce(local[:], stv, axis=mybir.AxisListType.X,
                                    op=mybir.AluOpType.add)

            # ---- AllReduce of [32,2] masked sums across cores -------------
            ar_in = dpool.tile([S, 2], f32)
            ar_out = dpool.tile([S, 2], f32)
            nc.gpsimd.dma_start(ar_in[:], local[:])
            nc.gpsimd.collective_compute(
                "AllReduce",
                mybir.AluOpType.add,
                replica_groups=[list(range(ncores))],
                ins=[ar_in.opt()],
                outs=[ar_out.opt()],
            )
            gst = glue.tile([S, 2], f32)
            nc.gpsimd.dma_start(gst[:], ar_out[:])

            # ---- fold stats into weights ----------------------------------
            mean = glue.tile([S, 1], f32)
            nc.vector.tensor_mul(mean[:], gst[:, 0:1], svsb[:, 2:3])
            ex2 = glue.tile([S, 1], f32)
            nc.vector.tensor_mul(ex2[:], gst[:, 1:2], svsb[:, 2:3])
            var = glue.tile([S, 1], f32)
            nc.vector.tensor_mul(var[:], mean[:], mean[:])
            nc.vector.tensor_sub(var[:], ex2[:], var[:])
            nc.vector.tensor_scalar_add(var[:], var[:], BN_EPS)
            recip = glue.tile([S, 1], f32)
            nc.vector.reciprocal(recip[:], var[:])
            rstd = glue.tile([S, 1], f32)
            nc.scalar.activation(rstd[:], recip[:],
                                 mybir.ActivationFunctionType.Sqrt)
            sg = glue.tile([S, 1], f32)
            nc.vector.tensor_mul(sg[:], rstd[:], svsb[:, 0:1])      # s = gamma*rstd
            tv = glue.tile([S, 1], f32)
            nc.vector.tensor_mul(tv[:], mean[:], sg[:])
            nc.vector.tensor_sub(tv[:], svsb[:, 1:2], tv[:])        # t = beta-mean*s
            # b1' = W1.T @ t + b1  (plain-fp32 matmul on the raw W1 copy)
            b1p = pspool1.tile([H, 1], f32)
            nc.tensor.matmul(b1p[:], w1rsb[:], tv[:], start=True,
                             stop=True, tile_position=(0, 0))
            b1f = glue.tile([H, 1], f32)
            nc.vector.tensor_add(b1f[:], b1p[:], b1sb[:])

            # broadcast b1' to [128,1] and s to [128,1] (4 copies)
            bias128 = wpool.tile([128, 1], f32)
            nc.sync.dma_start(bias128[0:H, :], b1f[:])
            nc.sync.dma_start(bias128[H:128, :], b1f[:])
            s4 = wpool.tile([128, 1], f32)
            for q in range(QS):
                nc.sync.dma_start(s4[32 * q:32 * q + 32, :], sg[:])
            # scale all four W1 copies in place: W1' = diag(s) @ W1
            nc.vector.tensor_scalar(w1sb[:], w1sb[:], s4[:], None,
                                    op0=mybir.AluOpType.mult)

            # ---- pass 2: the MLP ------------------------------------------
            relu = mybir.ActivationFunctionType.Relu
            ident = mybir.ActivationFunctionType.Identity
            order = list(range(keep)) + list(range(keep, TG))
            for g in order:
                if g in xtiles:
                    xt = xtiles[g]
                else:
                    xt = stpool.tile([128, MG * FD], fmm, tag="stream")
                    nc.sync.dma_start(xt[:], xview[g])
                omega = opool.tile([128, MG * FD], f32)
                for u in range(MG):
                    t = g * MG + u
                    xs = xt[:, FD * u:FD * u + FD]
                    psA = pspool.tile([128, FD], f32, tag="psA")
                    psB = pspool.tile([128, FD], f32, tag="psB")
                    # paired mm1: blockdiag(W1',W1') handles two quarters per
                    # column; outputs land exactly like the 4-way version
                    nc.tensor.matmul(psA[:], w1sb[0:2 * S, :], xs[0:2 * S, :],
                                     start=True, stop=True,
                                     tile_position=(0, 0))
                    nc.tensor.matmul(psB[:], w1sb[2 * S:128, :],
                                     xs[2 * S:128, :],
                                     start=True, stop=True,
                                     tile_position=(64, 0))
                    hA = hpool.tile([128, FD], f32, tag="hA")
                    hB = hpool.tile([128, FD], f32, tag="hB")
                    # relu(z + b1'): alternate engines to balance ACT/DVE
                    if t % 2 == 0:
                        nc.scalar.activation(hA[:], psA[:], relu,
                                             bias=bias128[:])
                        nc.vector.tensor_scalar(hB[:], psB[:], bias128[:], 0.0,
                                                op0=mybir.AluOpType.add,
                                                op1=mybir.AluOpType.max)
                    else:
                        nc.vector.tensor_scalar(hA[:], psA[:], bias128[:], 0.0,
                                                op0=mybir.AluOpType.add,
                                                op1=mybir.AluOpType.max)
                        nc.scalar.activation(hB[:], psB[:], relu,
                                             bias=bias128[:])
                    psC = pspool.tile([128, FD], f32, tag="psC")
                    for q in range(QS):
                        hs = (hA, hB)[q // 2]
                        o = q % 2
                        nc.tensor.matmul(psC[32 * q:32 * q + 32, :],
                                         w2sb[64 * o:64 * o + 64, :],
                                         hs[64 * o:64 * o + 64, :],
                                         start=True, stop=True,
                                         tile_position=(64 * o, 32 * q))
                    od = omega[:, FD * u:FD * u + FD]
                    if t % 3 == 2:
                        nc.vector.tensor_scalar(od, psC[:], b2sb[:], None,
                                                op0=mybir.AluOpType.add)
                    else:
                        nc.scalar.activation(od, psC[:], ident, bias=b2sb[:])
                nc.sync.dma_start(oview[g], omega[:])

    nc.compile()
    return nc


def _get_nc(ncores, rows):
    key = (ncores, rows)
    if key not in _NC_CACHE:
        _NC_CACHE[key] = build_nc(ncores, rows)
    return _NC_CACHE[key]


def _round_f32r(a):
    """Round fp32 array to float32r (low 12 mantissa bits dropped, RTN)."""
    if not USE_F32R:
        return a
    u = a.view(np.uint32)
    r = (u + np.uint32(0x800)) & np.uint32(0xFFFFF000)
    return r.view(np.float32)


def _tile_layout(xm, rows):
    """[rows, S] masked input -> [TG, 128, MG*FD] device tile layout."""
    T = rows // ST
    TG = T // MG
    # row r = ((g*MG + u)*QS + q)*FD + j ; partition p = q*32 + s
    v = xm.reshape(TG, MG, QS, FD, S)          # [g, u, q, j, s]
    v = v.transpose(0, 2, 4, 1, 3)             # [g, q, s, u, j]
    return np.ascontiguousarray(v).reshape(TG, 128, MG * FD)


def make_in_maps(stacks, mask, gamma, beta, W1, b1, W2, b2, ncores=NCORES,
                 rows=R_FULL):
    """Host-side prep: per-core input dicts (layout transforms only)."""
    stacks = np.asarray(stacks)
    mask = np.asarray(mask)
    cnt = max(float(np.asarray(mask, np.float64).sum()), 1.0)
    inv_cnt = np.float32(1.0 / np.float32(cnt))

    svec = np.zeros((S, 4), np.float32)
    svec[:, 0] = np.asarray(gamma, np.float32)
    svec[:, 1] = np.asarray(beta, np.float32)
    svec[:, 2] = inv_cnt

    w1np = np.asarray(W1, np.float32)
    bd = np.zeros((2 * S, 2 * H), np.float32)     # blockdiag(W1, W1)
    bd[:S, :H] = w1np
    bd[S:, H:] = w1np
    w1f = _round_f32r(np.tile(bd, (2, 1)))        # [128, 128]
    w2pad = np.zeros((64, 2 * HEADS), np.float32)
    w2pad[:, :HEADS] = np.asarray(W2, np.float32)
    w2t = np.tile(w2pad, (2, 1))                  # [128, 32]
    b1c = np.asarray(b1, np.float32).reshape(H, 1)
    b2t = np.zeros((128, 1), np.float32)
    for q in range(QS):
        b2t[32 * q:32 * q + HEADS, 0] = np.asarray(b2, np.float32)

    in_maps = []
    for c in range(ncores):
        x = np.asarray(stacks[c], np.float32).reshape(-1, S)[:rows]
        m = np.asarray(mask[c]).reshape(-1)[:rows]
        xm = _round_f32r(x * m[:, None].astype(np.float32))
        in_maps.append({
            "xmt": _tile_layout(xm, rows), "w1f": w1f, "w2t": w2t,
            "w1r": w1np, "svec": svec, "b1c": b1c, "b2t": b2t,
        })
    return in_maps


def assemble_output(results, mask, ncores=NCORES, rows=R_FULL):
    T = rows // ST
    TG = T // MG
    outs = []
    for c in range(ncores):
        o = results[c]["out"]                       # [TG, 128, MG*FD]
        o = o.reshape(TG, QS, 32, MG, FD)[:, :, :HEADS]   # [g, q, h, u, j]
        o = o.transpose(0, 3, 1, 4, 2)              # [g, u, q, j, h]
        o = np.ascontiguousarray(o).reshape(rows, HEADS)
        m = np.asarray(mask[c]).reshape(-1)[:rows]
        outs.append(o * m[:, None].astype(np.float32))
    return np.stack(outs)                           # [ncores, rows, 16]


def kernel(stacks, mask, gamma, beta, W1, b1, W2, b2):
    from concourse.bass_utils import run_bass_kernel_spmd

    nc = _get_nc(NCORES, R_FULL)
    in_maps = make_in_maps(stacks, mask, gamma, beta, W1, b1, W2, b2)
    res = run_bass_kernel_spmd(nc, in_maps, list(range(NCORES)))
    out = assemble_output(res.results, mask)
    return out.reshape(B, NN, NN, HEADS)
